# revision 14
# baseline (speedup 1.0000x reference)
"""Trainium2 Bass kernel for an 8-head cross-attention block.

Math (per reference):
    Q = video @ Wq[h]           [4096, 64]  per head
    K = text  @ Wk[h]           [1024, 64]
    V = text  @ Wv[h]           [1024, 64]
    att = softmax(Q @ K^T)      [4096, 1024]   (no scaling)
    y_h = att @ V               [4096, 64]
    out = concat_h(y_h) @ Wout + pos_enc(4096, 512)

Sharding: head-parallel over 8 NeuronCores. Core h owns head h and the
matching 64 rows of Wout (row-parallel); the all-reduce over cores and the
positional-encoding add happen on host during the gather.

Pipeline layout (v2 — restructured from the phase-serial baseline):
  * vt is DMA'd in four j-blocks on a second queue so Q-projection and the
    attention loop start ~4us in instead of waiting for the full 4MB.
  * Q/K projections run as col-tiled pairs (two M=64 matmuls in PE col
    groups 0-1/2-3 concurrently); E matmuls as row-tiled pairs (K=64,
    j-even on partitions 0-63, j-odd on 64-127) as in the baseline.
  * exp runs on ScalarE over [128,1024] PSUM tiles; E, Y, and the output
    projection are emitted interleaved per j-pair so TensorE never idles
    long enough to drop the HAM clock gate.
  * The softmax denominator comes free as a 65th row of att@V (ones column
    in V'); the [65,512] PSUM tile is drained in ONE DVE copy (y + fp16 den
    row), and each chunk's den does a tiny DRAM round-trip (transpose to
    [128,4]) + reciprocal off the critical path; 1/den is applied in the
    output-projection PSUM->SBUF copy (tensor_scalar, fused, no extra pass).
"""

import numpy as np

from concourse import bacc
import concourse.mybir as mybir
from concourse.tile import TileContext
from concourse.bass_utils import run_bass_kernel_spmd

N, M, D, H, DH = 4096, 1024, 512, 8, 64
P = 128
NC = 512          # n-chunk width (one j)
NJ = N // NC      # 8 n-chunks
NT = NJ // 2      # 4 j-pairs
DC = D // P       # 4 contraction chunks of 128
MT = M // P       # 8 key tiles of 128
F32 = mybir.dt.float32
FP16 = mybir.dt.float16
EXP = mybir.ActivationFunctionType.Exp
EXP_SHIFT = -12.0  # exp(E + shift): keeps exp in fp16 range; cancels in softmax
NCORES = 8

_CACHE: dict = {}
TRACE = False          # test harness can flip this before calling kernel()
LAST_RESULT = None     # BassKernelResults of the last run (for profiling)


def _body(tc, nc, vT, tT, wq, wk, wv, wo, out, dscr):
    with tc.tile_pool(name="const", bufs=1) as cp, \
         tc.tile_pool(name="pbuf", bufs=16) as pp, \
         tc.tile_pool(name="obuf", bufs=2) as op, \
         tc.tile_pool(name="ps_e", bufs=2, space="PSUM") as pe_pool, \
         tc.tile_pool(name="ps_y", bufs=2, space="PSUM") as py_pool, \
         tc.tile_pool(name="ps_m", bufs=2, space="PSUM") as pm_pool:

        tt_sb = cp.tile([P, DC * M], FP16, tag="tt")
        vt_sb = cp.tile([P, DC * N], FP16, tag="vt")
        wq_sb = cp.tile([P, DC * DH], FP16, tag="wq")
        wk_sb = cp.tile([P, DC * DH], FP16, tag="wk")
        wv_sb = cp.tile([P, DC * DH], FP16, tag="wv")
        wo_sb = cp.tile([DH, D], FP16, tag="wo")
        kt_sb = cp.tile([P, M], FP16, tag="kt")       # K^T duplicated on halves
        qt_sb = cp.tile([P, NT * NC], FP16, tag="qt")  # rows 0:64 j-even, 64:128 j-odd
        v3 = cp.tile([P, MT * (DH + 1)], FP16, tag="v3")
        y65 = cp.tile([DH + 1, NJ * NC], FP16, tag="y65")  # row 64 = fp16 den
        bias_sb = cp.tile([P, 1], F32, tag="bias")
        rsrc = cp.tile([P, N // P], FP16, tag="rsrc")
        rcf = cp.tile([P, N // P], F32, tag="rcf")
        rc_sb = cp.tile([P, N // P], F32, tag="rc")

        tt3 = tt_sb.rearrange("p (c m) -> p c m", c=DC)
        vt3 = vt_sb.rearrange("p (c n) -> p c n", c=DC)
        wq3 = wq_sb.rearrange("p (c e) -> p c e", c=DC)
        wk3 = wk_sb.rearrange("p (c e) -> p c e", c=DC)
        wv3 = wv_sb.rearrange("p (c e) -> p c e", c=DC)
        v3r = v3.rearrange("p (m e) -> p m e", e=DH + 1)

        # ---- input DMAs: spread issues across idle engine queues so the
        # critical tensors (tt, wk) are in flight within ~1us of preamble end
        nc.sync.dma_start(out=tt3[:, :, :], in_=tT.rearrange("(c p) m -> p c m", p=P))
        nc.scalar.dma_start(out=wk3[:, :, :], in_=wk.rearrange("(c p) e -> p c e", p=P))
        nc.scalar.dma_start(out=wv3[:, :, :], in_=wv.rearrange("(c p) e -> p c e", p=P))
        nc.sync.dma_start(out=wq3[:, :, :], in_=wq.rearrange("(c p) e -> p c e", p=P))
        nc.sync.dma_start(out=wo_sb[:, :], in_=wo[:, :])
        vTr = vT.rearrange("(c p) n -> p c n", p=P)
        for t in range(NT):
            nc.gpsimd.dma_start(
                out=vt3[:, :, t * 2 * NC:(t + 1) * 2 * NC],
                in_=vTr[:, :, t * 2 * NC:(t + 1) * 2 * NC])

        nc.vector.memset(v3r[:, :, DH], 1.0)
        nc.vector.memset(bias_sb[:, :], EXP_SHIFT)

        # ---- K proj: col-tiled pair (m-halves in PE col groups) ----
        psk = pm_pool.tile([P, NC], F32, tag="mm")
        for c in range(DC):
            nc.tensor.matmul(psk[0:DH, :], wk3[:, c, :], tt3[:, c, 0:512],
                             start=(c == 0), stop=(c == DC - 1))
            nc.tensor.matmul(psk[DH:P, :], wk3[:, c, :], tt3[:, c, 512:1024],
                             start=(c == 0), stop=(c == DC - 1))
        nc.vector.tensor_copy(out=kt_sb[0:DH, 0:512], in_=psk[0:DH, :])
        nc.vector.tensor_copy(out=kt_sb[DH:P, 0:512], in_=psk[0:DH, :])
        nc.vector.tensor_copy(out=kt_sb[0:DH, 512:1024], in_=psk[DH:P, :])
        nc.vector.tensor_copy(out=kt_sb[DH:P, 512:1024], in_=psk[DH:P, :])

        # ---- V proj: all 8 m-tiles into one PSUM bank ----
        psv = pm_pool.tile([P, NC], F32, tag="mm")
        for mt in range(MT):
            for c in range(DC):
                nc.tensor.matmul(psv[:, mt * DH:(mt + 1) * DH],
                                 tt3[:, c, mt * P:(mt + 1) * P], wv3[:, c, :],
                                 start=(c == 0), stop=(c == DC - 1))
        nc.vector.tensor_copy(out=v3r[:, :, 0:DH],
                              in_=psv.rearrange("p (m e) -> p m e", e=DH))

        # ---- Q proj per j-pair: col-tiled pair (j-even/j-odd) ----
        def q_proj(t):
            psq = pm_pool.tile([P, NC], F32, tag="mm")
            for c in range(DC):
                nc.tensor.matmul(psq[0:DH, :], wq3[:, c, :],
                                 vt3[:, c, (2 * t) * NC:(2 * t + 1) * NC],
                                 start=(c == 0), stop=(c == DC - 1))
                nc.tensor.matmul(psq[DH:P, :], wq3[:, c, :],
                                 vt3[:, c, (2 * t + 1) * NC:(2 * t + 2) * NC],
                                 start=(c == 0), stop=(c == DC - 1))
            nc.vector.tensor_copy(out=qt_sb[:, t * NC:(t + 1) * NC], in_=psq[:, :])

        q_proj(0)  # later j-pairs' Q projections are interleaved into the
        # attention loop so PE's in-order queue never head-blocks on vt DMAs

        # ---- attention + output projection, software-pipelined over t ----
        def emit_y_mms(p_list, y_ps, mts):
            # att@V for one chunk: accumulate over the listed m-tiles
            for mt in mts:
                pt = p_list[mt // 2]
                nc.tensor.matmul(y_ps[:, :], v3r[:, mt, :],
                                 pt[:, (mt % 2) * NC:(mt % 2 + 1) * NC],
                                 start=(mt == 0), stop=(mt == MT - 1))

        def drain_y(j, y_ps):
            jsl = slice(j * NC, (j + 1) * NC)
            nc.vector.tensor_copy(out=y65[:, jsl], in_=y_ps[:, :])
            # den chunk: DRAM round-trip transposes [1,512] -> [128,4]
            nc.gpsimd.dma_start(out=dscr[jsl], in_=y65[DH:DH + 1, jsl])
            nc.gpsimd.dma_start(
                out=rsrc[:, j * 4:(j + 1) * 4],
                in_=dscr[jsl].rearrange("(t p) -> p t", p=P))
            nc.vector.tensor_copy(out=rcf[:, j * 4:(j + 1) * 4],
                                  in_=rsrc[:, j * 4:(j + 1) * 4])
            nc.vector.reciprocal(rc_sb[:, j * 4:(j + 1) * 4],
                                 rcf[:, j * 4:(j + 1) * 4])

        out_r = out.rearrange("(g p) d -> p g d", p=P)  # [128, 32, 512]

        def out_proj(t):
            # output projection for chunks 2t, 2t+1 (needs rc of both)
            ot = op.tile([P, 8 * D], FP16, tag="ot")
            for jj in (2 * t, 2 * t + 1):
                for g in range(4):
                    po = pm_pool.tile([P, D], F32, tag="mm")
                    nt = jj * 4 + g
                    nc.tensor.matmul(
                        po[:, :],
                        y65[0:DH, jj * NC + g * P: jj * NC + (g + 1) * P],
                        wo_sb[:, :], start=True, stop=True)
                    osl = slice((nt - 8 * t) * D, (nt - 8 * t + 1) * D)
                    if t >= NT - 2 and g % 2 == 0:
                        # tail: scalar engine is done with exp — share the drain
                        nc.scalar.mul(ot[:, osl], po[:, :], rc_sb[:, nt:nt + 1])
                    else:
                        nc.vector.tensor_scalar_mul(
                            ot[:, osl], po[:, :], rc_sb[:, nt:nt + 1])
            nc.sync.dma_start(
                out=out_r[:, 8 * t:8 * (t + 1), :],
                in_=ot.rearrange("p (g d) -> p g d", d=D))

        prev_p = None   # p tiles of the previous j-pair: [ev0, od0, ev1, od1, ...]
        for t in range(NT):
            tsl = slice(t * NC, (t + 1) * NC)
            if prev_p is not None:
                y_ev = py_pool.tile([DH + 1, NC], F32, tag="y")
                y_od = py_pool.tile([DH + 1, NC], F32, tag="y")
            cur_p = []
            for i in range(4):
                # E tiles (mt pair 2i, 2i+1) for j-even and j-odd, row-tiled
                e_ev = pe_pool.tile([P, 2 * NC], F32, tag="e")
                e_od = pe_pool.tile([P, 2 * NC], F32, tag="e")
                for k in range(2):
                    mt = 2 * i + k
                    msl = slice(mt * P, (mt + 1) * P)
                    nc.tensor.matmul(e_ev[:, k * NC:(k + 1) * NC],
                                     kt_sb[0:DH, msl], qt_sb[0:DH, tsl],
                                     start=True, stop=True)
                    nc.tensor.matmul(e_od[:, k * NC:(k + 1) * NC],
                                     kt_sb[DH:P, msl], qt_sb[DH:P, tsl],
                                     start=True, stop=True)
                p_ev = pp.tile([P, 2 * NC], FP16, tag="p")
                p_od = pp.tile([P, 2 * NC], FP16, tag="p")
                nc.scalar.activation(p_ev[:, :], e_ev[:, :], EXP, bias=bias_sb[:, :])
                nc.scalar.activation(p_od[:, :], e_od[:, :], EXP, bias=bias_sb[:, :])
                cur_p.append((p_ev, p_od))
                # interleave att@V of the previous j-pair between E groups
                if prev_p is not None:
                    mts = (2 * i, 2 * i + 1)
                    emit_y_mms([a for a, b in prev_p], y_ev, mts)
                    emit_y_mms([b for a, b in prev_p], y_od, mts)
            if t + 1 < NT:
                q_proj(t + 1)
            if prev_p is not None:
                drain_y(2 * (t - 1), y_ev)
                drain_y(2 * (t - 1) + 1, y_od)
            if t >= 2:
                out_proj(t - 2)
            prev_p = cur_p

        # tail: att@V + drain for the last j-pair, then remaining projections
        y_ev = py_pool.tile([DH + 1, NC], F32, tag="y")
        y_od = py_pool.tile([DH + 1, NC], F32, tag="y")
        emit_y_mms([a for a, b in prev_p], y_ev, range(MT))
        emit_y_mms([b for a, b in prev_p], y_od, range(MT))
        drain_y(2 * (NT - 1), y_ev)
        drain_y(2 * (NT - 1) + 1, y_od)
        out_proj(NT - 2)
        out_proj(NT - 1)


def _build():
    nc = bacc.Bacc("TRN2", target_bir_lowering=False, debug=False)
    vT = nc.dram_tensor("vT", [D, N], FP16, kind="ExternalInput")
    tT = nc.dram_tensor("tT", [D, M], FP16, kind="ExternalInput")
    wq = nc.dram_tensor("wq", [D, DH], FP16, kind="ExternalInput")
    wk = nc.dram_tensor("wk", [D, DH], FP16, kind="ExternalInput")
    wv = nc.dram_tensor("wv", [D, DH], FP16, kind="ExternalInput")
    wo = nc.dram_tensor("wo", [DH, D], FP16, kind="ExternalInput")
    out = nc.dram_tensor("out", [N, D], FP16, kind="ExternalOutput")
    dscr = nc.dram_tensor("dscr", [N], FP16)
    with TileContext(nc) as tc:
        _body(tc, nc, vT[:, :], tT[:, :], wq[:, :], wk[:, :], wv[:, :],
              wo[:, :], out[:, :], dscr[:])
    nc.compile()
    return nc


def _pos_encoding():
    # Mirror the reference's jnp ops bit-for-bit (numpy's f32 sin/exp differ
    # by enough ULPs to dominate the error budget at pos/freq ~ 4e3).
    import jax
    import jax.numpy as jnp
    with jax.default_device(jax.devices("cpu")[0]):
        pos = jnp.arange(N, dtype=jnp.float32)
        freq = jnp.exp(
            (jnp.arange(D // 2, dtype=jnp.float32) / D)
            * jnp.log(jnp.float32(10000.0)))
        x = pos[:, None] / freq
        pe = jnp.stack((jnp.sin(x), jnp.cos(x)), axis=-1)
        return np.asarray(pe.reshape(N, D), dtype=np.float32)


def _fp16(a):
    return np.ascontiguousarray(np.asarray(a, dtype=np.float32).astype(np.float16))


def kernel(video_features, text_features, Wq, Wk, Wv, Wout):
    global LAST_RESULT
    if "nc" not in _CACHE:
        _CACHE["nc"] = _build()
        _CACHE["pe"] = _pos_encoding()
    nc = _CACHE["nc"]

    vT = _fp16(np.asarray(video_features, dtype=np.float32).T)
    tT = _fp16(np.asarray(text_features, dtype=np.float32).T)
    Wq = np.asarray(Wq, dtype=np.float32)
    Wk = np.asarray(Wk, dtype=np.float32)
    Wv = np.asarray(Wv, dtype=np.float32)
    Wout = np.asarray(Wout, dtype=np.float32)

    in_maps = []
    for h in range(NCORES):
        in_maps.append({
            "vT": vT,
            "tT": tT,
            "wq": _fp16(Wq[h]),
            "wk": _fp16(Wk[h]),
            "wv": _fp16(Wv[h]),
            "wo": _fp16(Wout[h * DH:(h + 1) * DH, :]),
        })
    res = run_bass_kernel_spmd(nc, in_maps, list(range(NCORES)), trace=TRACE)
    LAST_RESULT = res
    acc = res.results[0]["out"].astype(np.float32)
    for h in range(1, NCORES):
        acc = acc + res.results[h]["out"].astype(np.float32)
    return (acc + _CACHE["pe"]).astype(np.float32)
